# revision 12
# baseline (speedup 1.0000x reference)
"""Trainium2 Bass kernel for nn_MultiInfAffine.

Math (reference):
    mu_n = mus / ||mus||_D                          [L=6, D=16, K=64]
    t    = <x, mu_n>                                 per (l, n, k)
    d    = arccos(clip(t))
    cost = 0.5 d^2 + alpha
    mc_l = 0.1 * ln sum_k exp(-cost/0.1)
    F    = recurrence over l:  F = wv_l relu(F) + (1-wv_l) mc_l,  wv = exp(-ws^2)
    out  = 0.1 * ln(1 + exp(-10 F))

Device chain per element (2 instructions total):
    z = 1 + t                      -- folded into the inner-product matmul via an
                                      appended ones-dimension (contract = 17)
    h = ((((z+a1)z+a2)z+a3)z+a4)z  -- ONE fused custom DVE op (8 ALU stages)
    E = DErf(sig*h + tau)          -- ONE ACT pass = (2/sqrt(pi)) exp(-G^2), bf16
  where G(z) = sig*h + tau is a degree-5 fit of sqrt(5*(arccos(z-1)^2 - w)), so
  E = (2/sqrt(pi)) e^{5w} exp(-5 d^2).  The constants e^{-5w} and sqrt(pi)/2 are
  folded into the reduction weights exp(-10 alpha).

Reduction WITHOUT any transpose/staging: for each 128-point chunk, the E-block
[128 comps, 128 points] is loaded as the matmul STATIONARY and the weights
[128 comps, 6 layers] stream as moving data, accumulating (over the 3 partition
planes) S^T[128 points, 6] directly into PSUM -- already in tail layout.
The tail (Ln + 6-step recurrence + smooth-min) runs per group of 64 chunks.

Layout: 128 SBUF partitions = 2 layers x 64 components ("plane" g covers layers
2g, 2g+1; 3 planes). Points stream along the free axis in 512-column subtiles.
Only the DErf activation table is used in the main loop -> no table reloads.
"""

import numpy as np
import ml_dtypes

import concourse.bacc as bacc
import concourse.tile as tile
from concourse import mybir
from concourse.bass_utils import run_bass_kernel_spmd

# ---------------------------------------------------------------------------
# Custom DVE op: monic degree-5 Horner  h = ((((z+a1)z+a2)z+a3)z+a4)z
# ---------------------------------------------------------------------------
import concourse.dve_ops as dve_ops
from concourse.dve_spec import (
    Spec, Src0, C0, C1, C2, C3, lower, _spill_c3_to_src1, _has_src1,
)
from concourse.dve_uop import DveOpSpec


def _q5_ref(in0, in1, s0, s1, imm2):
    c3 = np.asarray(in1, np.float32).reshape(in0.shape[0], -1)[:, :1]
    z = in0.astype(np.float32)
    return ((((z + np.float32(s0)) * z + np.float32(s1)) * z + np.float32(imm2))
            * z + c3) * z


def _register_q5():
    name = "Q5_HORNER_ANT"
    if name in dve_ops._SUB_OPCODE_FOR_NAME:
        return next(o for o in dve_ops.OPS if o.name == name)
    body = ((((Src0 + C0) * Src0 + C1) * Src0 + C2) * Src0 + C3) * Src0
    spec = Spec(body=_spill_c3_to_src1(body), reference=_q5_ref)
    row = max(dve_ops._SUB_OPCODE_FOR_NAME.values()) + 1
    dve_ops._SUB_OPCODE_FOR_NAME[name] = row
    shas = {}
    for ver in ("v3", "v4"):
        sp = DveOpSpec(name=name, opcode=row, uops=lower(spec, ver=ver),
                       rd1_en=_has_src1(spec))
        shas[ver] = sp.sha(ver)
    op = dve_ops.DveOp(name, spec, subdim=False, uops_sha=shas)
    dve_ops.OPS.append(op)
    dve_ops.CUSTOM_DVE_SPECS[name] = spec
    return op


Q5_OP = _register_q5()

# ---------------------------------------------------------------------------
# Problem + tiling constants
# ---------------------------------------------------------------------------
N, D, L, K = 250000, 16, 6, 64
NCORES = 8
NPC = N // NCORES        # 31250 true points per core

SC = 512                 # points per subtile (1 PSUM bank per plane)
NSUB = 62                # subtiles per core
NPAD = SC * NSUB         # 31744 padded points per core
CPS = SC // 128          # 4 point-chunks per subtile
NCHUNK = NPAD // 128     # 248 chunks per core
GRP = 21                 # subtiles per tail group (84 chunks -> 1 PSUM bank)

# G(z) ~ sqrt(5*(arccos(z-1)^2 - W0)), degree-5 fit; E = 2/sqrt(pi) exp(-G^2).
# coeffs c5..c0 (highest first) + w; refreshed by fit/polish runs.
POLY = np.array([-0.04856858, 0.32470551, -1.03614173, 1.83704212,
                 -3.45642527, 6.66012459], np.float64)
W0 = -1.1975375009395908

# Path-B (ACT Square + 2 GPSIMD Horner steps):
#   u = (PB_S z + PB_B0)^2 ; G = PB_SIG*(((u+PB_A)u+PB_B)u) + PB_TAU
# placeholder values; refreshed by fit_pathb.py. PB_EVERY=k routes every k-th
# subtile through path B (0 = disabled).
PATHB = None
PB_EVERY = 3

F32 = mybir.dt.float32
F32R = mybir.dt.float32r
BF16 = mybir.dt.bfloat16
AF = mybir.ActivationFunctionType
ALU = mybir.AluOpType


def _build(wv=None, repeat=1):
    """Build the per-core Bass program. wv: np.float32[L] = exp(-ws^2)."""
    assert wv is not None

    c5, c4, c3, c2, c1, c0 = [float(v) for v in POLY]
    A1, A2, A3, A4 = c4 / c5, c3 / c5, c2 / c5, c1 / c5
    SIG, TAU = c5, c0

    A = [float(wv[l]) for l in range(L)]
    B = [float((1.0 - wv[l]) * 0.1) for l in range(L)]
    pb = None if (PATHB is None or PB_EVERY == 0) else [float(v) for v in PATHB]

    nc = bacc.Bacc()

    xst = nc.dram_tensor("xst", [D + 1, NPAD], F32R, kind="ExternalInput")
    mu = nc.dram_tensor("mu", [D + 1, 3, 128], F32R, kind="ExternalInput")
    ow = nc.dram_tensor("ow", [128, 3, 6], BF16, kind="ExternalInput")
    fout = nc.dram_tensor("fout", [NPAD], F32, kind="ExternalOutput")

    with tile.TileContext(nc) as tc:
        with (
            tc.tile_pool(name="singles", bufs=1) as singles,
            tc.tile_pool(name="xs", bufs=4) as xpool,
            tc.tile_pool(name="zpsum", bufs=2, space="PSUM") as zpool,
            tc.tile_pool(name="spsum", bufs=2, space="PSUM") as spool,
            tc.tile_pool(name="h", bufs=3) as hpool,
            tc.tile_pool(name="e", bufs=3) as epool,
            tc.tile_pool(name="u", bufs=2) as upool,
            tc.tile_pool(name="tail", bufs=2) as tailpool,
        ):
            mu_sb = singles.tile([D + 1, 3, 128], F32R)
            nc.sync.dma_start(out=mu_sb[:], in_=mu[:])
            ow_sb = singles.tile([128, 3, 6], BF16)
            nc.sync.dma_start(out=ow_sb[:], in_=ow[:])
            a4_sb = singles.tile([128, 1], F32)
            nc.vector.memset(a4_sb[:], A4)
            tau_sb = singles.tile([128, 1], F32)
            nc.vector.memset(tau_sb[:], TAU)
            if pb is not None:
                b0_sb = singles.tile([128, 1], F32)
                nc.vector.memset(b0_sb[:], pb[1])
                taub_sb = singles.tile([128, 1], F32)
                nc.vector.memset(taub_sb[:], pb[5])
            else:
                b0_sb = taub_sb = None

            args = (nc, tc, A, B, A1, A2, A3, SIG, pb,
                    xst, fout, mu_sb, ow_sb, a4_sb, tau_sb, b0_sb, taub_sb,
                    xpool, zpool, spool, hpool, epool, upool, tailpool)
            if repeat > 1:
                with tc.For_i(0, repeat, 1):
                    _emit_body(*args)
            else:
                _emit_body(*args)

    nc.compile()
    return nc


def _emit_body(nc, tc, A, B, A1, A2, A3, SIG, pb,
               xst, fout, mu_sb, ow_sb, a4_sb, tau_sb, b0_sb, taub_sb,
               xpool, zpool, spool, hpool, epool, upool, tailpool):
    fout_pt = fout[:].rearrange("(t p) -> p t", p=128)  # [128, NCHUNK]

    acc = {}    # group -> (psum tile, n_chunks)
    zs = {}     # subtile -> z psum tile
    hs = {}     # subtile -> (h sbuf tile, is_path_b)

    def is_b(s):
        return pb is not None and s % PB_EVERY == PB_EVERY - 1

    def emit_P(s):
        c0 = s * SC
        xs_t = xpool.tile([D + 1, SC], F32R, tag="xs")
        nc.sync.dma_start(out=xs_t[:], in_=xst[:, c0:c0 + SC])
        z_t = zpool.tile([128, 3, SC], F32, tag="z")
        for g in range(3):
            nc.tensor.matmul(z_t[:, g, :], mu_sb[:, g, :], xs_t[:])
        zs[s] = z_t

    def emit_M(s):
        z_t = zs.pop(s)
        h_t = hpool.tile([128, 3, SC], F32, tag="h")
        if is_b(s):
            # path B: ACT Square -> pool Horner (monic cubic in u)
            u_t = upool.tile([128, 3, SC], F32, tag="u")
            nc.scalar.activation(u_t[:], z_t[:], AF.Square, scale=pb[0],
                                 bias=b0_sb[:])
            nc.gpsimd.tensor_scalar_add(h_t[:], u_t[:], pb[2])
            nc.gpsimd.tensor_mul(h_t[:], h_t[:], u_t[:])
            nc.gpsimd.tensor_scalar_add(h_t[:], h_t[:], pb[3])
            nc.gpsimd.tensor_mul(h_t[:], h_t[:], u_t[:])
        else:
            nc.vector._custom_dve(Q5_OP, out=h_t[:], in0=z_t[:],
                                  in1=a4_sb[:], s0=A1, s1=A2, imm2=A3)
        hs[s] = h_t

    def emit_C(s):
        h_t = hs.pop(s)
        e_t = epool.tile([128, 3, SC], BF16, tag="e")
        if is_b(s):
            nc.scalar.activation(e_t[:], h_t[:], AF.Derivative_Erf,
                                 scale=pb[4], bias=taub_sb[:])
        else:
            nc.scalar.activation(e_t[:], h_t[:], AF.Derivative_Erf, scale=SIG,
                                 bias=tau_sb[:])
        # transposed reduction: E-chunk stationary, weights moving ->
        # S^T[128 points, 6 layers] accumulated over the 3 planes.
        gr = s // GRP
        if gr not in acc:
            nch = min(NSUB - gr * GRP, GRP) * CPS
            a_t = spool.tile([128, nch, 6], F32, tag="acc", name=f"acc{gr}")
            acc[gr] = (a_t, nch)
        a_t, _ = acc[gr]
        for c in range(CPS):
            col = (s - gr * GRP) * CPS + c
            for g in range(3):
                nc.tensor.matmul(a_t[:, col, :],
                                 e_t[:, g, 128 * c:128 * (c + 1)],
                                 ow_sb[:, g, :],
                                 start=(g == 0), stop=(g == 2))
        if s % GRP == GRP - 1 or s == NSUB - 1:
            emit_tail(gr)

    def emit_tail(gr):
        a_t, nch = acc.pop(gr)
        mc = tailpool.tile([128, nch, 6], F32, tag="mc")
        nc.scalar.activation(mc[:], a_t[:], AF.Ln)
        for l in range(L):
            nc.gpsimd.tensor_scalar_mul(mc[:, :, l], mc[:, :, l], B[l])
        f_t = tailpool.tile([128, nch], F32, tag="f")
        nc.gpsimd.tensor_copy(f_t[:], mc[:, :, 0])
        for l in range(1, L):
            # F = A_l * relu(F) + mc_l  (two pool ops: fused max+mul, then add)
            nc.gpsimd.tensor_scalar(out=f_t[:], in0=f_t[:], scalar1=0.0,
                                    scalar2=A[l], op0=ALU.max, op1=ALU.mult)
            nc.gpsimd.tensor_add(f_t[:], f_t[:], mc[:, :, l])
        nc.scalar.activation(f_t[:], f_t[:], AF.Exp, scale=-10.0)
        nc.scalar.activation(f_t[:], f_t[:], AF.Ln, bias=1.0)
        nc.gpsimd.tensor_scalar_mul(f_t[:], f_t[:], 0.1)
        t0 = gr * GRP * CPS
        nc.sync.dma_start(out=fout_pt[:, t0:t0 + nch], in_=f_t[:])

    # software-pipelined emission: producer / middle / consumer offset by one
    # subtile each so no engine queue waits on a same-subtile round trip.
    for i in range(NSUB + 2):
        if i < NSUB:
            emit_P(i)
        if 0 <= i - 2:
            emit_C(i - 2)
        if 0 <= i - 1 < NSUB:
            emit_M(i - 1)


def _host_prep(xs, mus, alphas, ws):
    """Returns (shared inputs dict, list of per-core xst arrays, wv)."""
    mus = np.asarray(mus, np.float32)
    alphas = np.asarray(alphas, np.float32)
    ws = np.asarray(ws, np.float32)
    xs = np.asarray(xs, np.float32)

    mu_n = mus / np.linalg.norm(mus, axis=1, keepdims=True)  # [L, D, K]
    # mu layout: [17, 3, 128]; column j of plane g is (layer 2g + j//64, k = j%64)
    mu_aug = np.zeros((D + 1, 3, 128), np.float32)
    for g in range(3):
        for half in range(2):
            layer = 2 * g + half
            mu_aug[:D, g, 64 * half:64 * half + 64] = mu_n[layer]
    mu_aug[D, :, :] = 1.0  # z = 1 + t

    # reduction weights: sqrt(pi)/2 * e^{-10 alpha - 5 W0} (row k, plane g, col l)
    ow = np.zeros((128, 3, 6), np.float32)
    wfac = float(np.sqrt(np.pi) / 2.0 * np.exp(-5.0 * W0))
    for g in range(3):
        for half in range(2):
            layer = 2 * g + half
            ow[64 * half:64 * half + 64, g, layer] = (
                wfac * np.exp(-10.0 * alphas[layer].astype(np.float64))
            ).astype(np.float32)
    ow = ow.astype(ml_dtypes.bfloat16)

    wv = np.exp(-ws.astype(np.float32) ** 2).astype(np.float32)

    n = xs.shape[0]
    per = n // NCORES
    xst_list = []
    for c in range(NCORES):
        shard = xs[c * per:(c + 1) * per]
        aug = np.ones((shard.shape[0], D + 1), np.float32)
        aug[:, :D] = shard
        pad = np.zeros((NPAD, D + 1), np.float32)
        pad[:, D] = 1.0  # pad points: x = 0 -> z = 1, harmless
        pad[:shard.shape[0]] = aug
        xst_list.append(np.ascontiguousarray(pad.T))  # [17, NPAD]
    return {"mu": mu_aug, "ow": ow}, xst_list, wv


def prepare(xs, mus, alphas, ws, repeat=1):
    """Build the Bass program and per-core input maps."""
    shared, xst_list, wv = _host_prep(xs, mus, alphas, ws)
    nc = _build(wv=wv, repeat=repeat)
    in_maps = [dict(shared, xst=xst_list[c]) for c in range(NCORES)]
    return nc, in_maps


def kernel(xs, mus, alphas, ws, trace=False, tmpdir=None):
    nc, in_maps = prepare(xs, mus, alphas, ws)
    res = run_bass_kernel_spmd(
        nc, in_maps, core_ids=list(range(NCORES)), trace=trace, tmpdir=tmpdir
    )
    per = N // NCORES
    out = np.concatenate([res.results[c]["fout"][:per] for c in range(NCORES)])
    kernel.last_results = res
    return out.astype(np.float32)
